# revision 35
# baseline (speedup 1.0000x reference)
"""Causal single-head attention (B=4, S=4096, D=512, fp32) on 8 Trainium2 cores.

Sharding: core c -> (batch b = c//2, parity p = c%2). Each core handles the 16
query tiles {2m+p} of its batch (parity-interleaved for causal balance; every
core runs the identical SPMD program, parity lives only in input data).

All heavy matmuls run as fp8-e4m3 DoubleRow (contracts 256/instr at 0.5
cyc/row vs bf16's 128 at 1.0 = 4x per MAC), with fp8+fp8-residual operand
splits recovering bf16-level precision where softmax is sensitive:
  C  = Wq^T Wk                 bf16 matmul; stored fp8 + residual (ct8x/ct8rx)
  A  = C^T Xq^T                3-term DR (c8*xq8 + c8r*xq8 + c8*xq8r ~ exact);
                               stored fp8 + residual (a8x/a8rx) + bf16 (slot 0)
  V  = X Wv^T                  tiles 0-3 bf16; tiles 4-31 plain DR (x8*wv8);
                               stored fp8 (vt8) + residual (vt8r, tiles 0-7)
  S^T = X^T A  per k-tile      3-term DR: x8*A8 + x8r*A8 + x8*A8r with
                               cross-d-tile ko-pairing -> bf16-equivalent
                               precision at 0.75x bf16 cost; slot 0 pure bf16
  P   = exp(S*scale - 4.5)     ScalarE -> fp8 (slot 0: bf16); causal masking
                               via 0/1 multiplies on the 4 diagonal-band
                               128x128 blocks per slot (GpSimd)
  out = P V / P.1              DR AV (+ V-residual term for slots 0-1, i.e.
                               rows < 1024) + ones-matmul denominator
Query rows < 512 (slot 0) take an all-bf16 path and rows < 1024 keep the
V-storage residual, because low-entropy early softmax rows amplify fp8 noise.
Input DMAs are chunked in first-consumer order; output is DMA'd bf16 and
upcast on host. Fits the TimelineSim cost model: matmul cost = out-free-size
x cycles/row, DoubleRow 0.5 cyc/row, PE p-state ramp warmed by scratch MMs.
"""

import numpy as np
import ml_dtypes

B, S, D = 4, 4096, 512
P = 128
SQ = S // 2          # local query rows per core
SCALE = float(1.0 / np.sqrt(D))
SHIFT = 4.5          # exp shift: keeps P in fp8-e4m3 range (max score ~8.4)
bf16 = ml_dtypes.bfloat16
f8 = ml_dtypes.float8_e4m3

_BUILT = None
TRACE = False        # set True from a test harness to collect an NTFF trace
LAST_RESULT = None   # BassKernelResults of the most recent run (for profiling)


def _build_nc():
    import concourse.bass as bass
    import concourse.tile as tile
    from concourse import bacc, mybir

    BF = mybir.dt.bfloat16
    F8 = mybir.dt.float8e4
    F32 = mybir.dt.float32
    Exp = mybir.ActivationFunctionType.Exp
    DR = mybir.MatmulPerfMode.DoubleRow

    nc = bacc.Bacc(trn_type="TRN2", name="causal_attn_dr")
    xp2_d = nc.declare_dram_parameter("xp2", [2, P, 2, S], F8, False)
    xp2r_d = nc.declare_dram_parameter("xp2r", [2, P, 2, S], F8, False)
    xt16_d = nc.declare_dram_parameter("xt16", [4, P, 512], BF, False)
    xq8x_d = nc.declare_dram_parameter("xq8x", [2, P, 2, SQ], F8, False)
    xq8rx_d = nc.declare_dram_parameter("xq8rx", [2, P, 2, SQ], F8, False)
    wq_d = nc.declare_dram_parameter("wq", [4, P, D], BF, False)
    wk_d = nc.declare_dram_parameter("wk", [4, P, D], BF, False)
    wv8x_d = nc.declare_dram_parameter("wv8x", [2, P, 2, D], F8, False)
    wvt16_d = nc.declare_dram_parameter("wvt16", [4, P, D], BF, False)
    mask8_d = nc.declare_dram_parameter("mask8", [P, 2, P], F8, False)
    mask16_d = nc.declare_dram_parameter("mask16", [P, 2, P], BF, False)
    out_d = nc.declare_dram_parameter("out", [SQ, D], BF, True)

    with tile.TileContext(nc) as tc:
        with (
            tc.tile_pool(name="persist", bufs=1) as persist,
            tc.tile_pool(name="ptp", bufs=28) as ptp,
            tc.tile_pool(name="pt16p", bufs=4) as pt16p,
            tc.tile_pool(name="outsp", bufs=6) as outsp,
            tc.tile_pool(name="smallp", bufs=4) as smallp,
            tc.tile_pool(name="qkv_ps", bufs=3, space="PSUM") as qkv_ps,
            tc.tile_pool(name="st_ps", bufs=3, space="PSUM") as st_ps,
            tc.tile_pool(name="av_ps", bufs=2, space="PSUM") as av_ps,
        ):
            # ---- persistent inputs, chunked in first-consumer order ----
            wq16, wk16, xt16, wvt16 = [], [], [], []
            for d in range(4):
                t = persist.tile([P, D], BF, name=f"wq{d}", tag=f"wq{d}")
                nc.sync.dma_start(out=t, in_=wq_d[d])
                wq16.append(t)
            for d in range(4):
                t = persist.tile([P, D], BF, name=f"wk{d}", tag=f"wk{d}")
                nc.sync.dma_start(out=t, in_=wk_d[d])
                wk16.append(t)
            for d in range(4):
                t = persist.tile([P, 512], BF, name=f"xt16_{d}", tag=f"xt16_{d}")
                nc.sync.dma_start(out=t, in_=xt16_d[d])
                xt16.append(t)
            for d in range(4):
                t = persist.tile([P, D], BF, name=f"wvt{d}", tag=f"wvt{d}")
                nc.sync.dma_start(out=t, in_=wvt16_d[d])
                wvt16.append(t)
            # A(0) inputs: q-block 0 slices only
            xq8x = [persist.tile([P, 2, SQ], F8, name=f"xq8x{i}", tag=f"xq8x{i}")
                    for i in range(2)]
            xq8rx = [persist.tile([P, 2, SQ], F8, name=f"xq8rx{i}",
                                  tag=f"xq8rx{i}") for i in range(2)]
            for i in range(2):
                nc.sync.dma_start(out=xq8x[i][:, :, 0:D], in_=xq8x_d[i][:, :, 0:D])
            for i in range(2):
                nc.sync.dma_start(out=xq8rx[i][:, :, 0:D],
                                  in_=xq8rx_d[i][:, :, 0:D])
            mask16 = persist.tile([P, 2, P], BF, name="mask16", tag="mask16")
            nc.sync.dma_start(out=mask16, in_=mask16_d[:])
            # fp8 X for kv1 / slot1 (k < 1024 chunks first)
            xp2 = [persist.tile([P, 2, S], F8, name=f"xp2_{dp}", tag=f"xp2_{dp}")
                   for dp in range(2)]
            xp2r = [persist.tile([P, 2, S], F8, name=f"xp2r_{dp}",
                                 tag=f"xp2r_{dp}") for dp in range(2)]
            wv8x = []
            for dp in range(2):
                nc.sync.dma_start(out=xp2[dp][:, :, 0:1024],
                                  in_=xp2_d[dp][:, :, 0:1024])
            for dp in range(2):
                t = persist.tile([P, 2, D], F8, name=f"wv8x{dp}", tag=f"wv8x{dp}")
                nc.sync.dma_start(out=t, in_=wv8x_d[dp])
                wv8x.append(t)
            mask8 = persist.tile([P, 2, P], F8, name="mask8", tag="mask8")
            nc.sync.dma_start(out=mask8, in_=mask8_d[:])
            for dp in range(2):
                nc.sync.dma_start(out=xp2r[dp][:, :, 0:1024],
                                  in_=xp2r_d[dp][:, :, 0:1024])
            # A(1) inputs
            for i in range(2):
                nc.sync.dma_start(out=xq8x[i][:, :, D:2 * D],
                                  in_=xq8x_d[i][:, :, D:2 * D])
            for i in range(2):
                nc.sync.dma_start(out=xq8rx[i][:, :, D:2 * D],
                                  in_=xq8rx_d[i][:, :, D:2 * D])
            # k in [1024, 2048) for kv2-3 / slots 2-3
            for dp in range(2):
                nc.sync.dma_start(out=xp2[dp][:, :, 1024:2048],
                                  in_=xp2_d[dp][:, :, 1024:2048])
            for dp in range(2):
                nc.sync.dma_start(out=xp2r[dp][:, :, 1024:2048],
                                  in_=xp2r_d[dp][:, :, 1024:2048])
            # remaining A q-blocks
            for i in range(2):
                nc.sync.dma_start(out=xq8x[i][:, :, 2 * D:], in_=xq8x_d[i][:, :, 2 * D:])
            for i in range(2):
                nc.sync.dma_start(out=xq8rx[i][:, :, 2 * D:],
                                  in_=xq8rx_d[i][:, :, 2 * D:])
            # k in [2048, 4096)
            for dp in range(2):
                nc.sync.dma_start(out=xp2[dp][:, :, 2048:], in_=xp2_d[dp][:, :, 2048:])
            for dp in range(2):
                nc.sync.dma_start(out=xp2r[dp][:, :, 2048:],
                                  in_=xp2r_d[dp][:, :, 2048:])

            # PE warmup: ramp the p-state clock while input DMAs stream in
            scratch = persist.tile([P, 512], BF, name="scratch", tag="scratch")
            nc.vector.memset(scratch, 0.0)
            ones8 = persist.tile([P, 2, 1], F8, name="ones8", tag="ones8")
            nc.vector.memset(ones8, 1.0)
            ones16 = persist.tile([P, 1], BF, name="ones16", tag="ones16")
            nc.vector.memset(ones16, 1.0)
            nshift = persist.tile([P, 1], F32, name="nshift", tag="nshift")
            nc.vector.memset(nshift, -SHIFT)

            for wi in range(8):
                wps = qkv_ps.tile([P, 512], F32, name="wps", tag="qkv_ps")
                nc.tensor.matmul(wps, lhsT=scratch[:, :P], rhs=scratch,
                                 start=True, stop=True)

            ct8x = [persist.tile([P, 2, D], F8, name=f"ct8x{i}", tag=f"ct8x{i}")
                    for i in range(2)]
            ct8rx = [persist.tile([P, 2, D], F8, name=f"ct8rx{i}",
                                  tag=f"ct8rx{i}") for i in range(2)]

            def emit_c():
                # C[dp, d] = sum_o Wq[o, dp] Wk[o, d]  (bf16 matmul, fp8+resid store)
                for dp in range(4):
                    ps = qkv_ps.tile([P, D], F32, name="c_ps", tag="qkv_ps")
                    for ot in range(4):
                        nc.tensor.matmul(ps,
                                         lhsT=wq16[ot][:, P * dp:P * (dp + 1)],
                                         rhs=wk16[ot], start=(ot == 0),
                                         stop=(ot == 3))
                    c8 = ct8x[dp // 2]
                    nc.vector.tensor_copy(out=c8[:, dp % 2, :], in_=ps)
                    nc.vector.tensor_sub(out=ct8rx[dp // 2][:, dp % 2, :],
                                         in0=ps, in1=c8[:, dp % 2, :])

            # persistent on-chip products
            a8x = [[persist.tile([P, 2, D], F8, name=f"a8x{dp}_{qb}",
                                 tag=f"a8x{dp}_{qb}") for qb in range(4)]
                   for dp in range(2)]
            a8rx = [[persist.tile([P, 2, D], F8, name=f"a8rx{dp}_{qb}",
                                  tag=f"a8rx{dp}_{qb}") for qb in range(4)]
                    for dp in range(2)]
            at16 = [persist.tile([P, D], BF, name=f"at16_{dt}", tag=f"at16_{dt}")
                    for dt in range(4)]
            vt8 = [persist.tile([P, 2, D], F8, name=f"vt8_{tp}", tag=f"vt8_{tp}")
                   for tp in range(16)]
            vt8r = [persist.tile([P, 2, D], F8, name=f"vt8r_{tp}", tag=f"vt8r_{tp}")
                    for tp in range(4)]
            vt16 = [persist.tile([P, D], BF, name=f"vt16_{sv}", tag=f"vt16_{sv}")
                    for sv in range(4)]

            def emit_a(qb):
                # A[d, q] = sum_dp C[dp, d] Xq^T[dp, q]
                # 3-term DoubleRow: c8*xq8 + c8r*xq8 + c8*xq8r ~= bf16 exact
                qsl = slice(D * qb, D * (qb + 1))
                for dt in range(4):
                    ps = qkv_ps.tile([P, D], F32, name="a_ps", tag="qkv_ps")
                    i = 0
                    for dpp in range(2):
                        for cs, xs in ((ct8x, xq8x), (ct8rx, xq8x),
                                       (ct8x, xq8rx)):
                            nc.tensor.matmul(
                                ps, lhsT=cs[dpp][:, :, P * dt:P * (dt + 1)],
                                rhs=xs[dpp][:, :, qsl],
                                start=(i == 0), stop=(i == 5), perf_mode=DR)
                            i += 1
                    a8 = a8x[dt // 2][qb]
                    nc.vector.tensor_copy(out=a8[:, dt % 2, :], in_=ps)
                    nc.vector.tensor_sub(out=a8rx[dt // 2][qb][:, dt % 2, :],
                                         in0=ps, in1=a8[:, dt % 2, :])
                    if qb == 0:
                        nc.scalar.copy(out=at16[dt], in_=ps)

            def emit_kv(kb):
                # V[s, o] = sum_d X[s, d] Wv[o, d]
                for sv in range(4 * kb, 4 * kb + 4):
                    ps = qkv_ps.tile([P, D], F32, name="v_ps", tag="qkv_ps")
                    if kb == 0:
                        for ot in range(4):
                            nc.tensor.matmul(
                                ps, lhsT=xt16[ot][:, P * sv:P * (sv + 1)],
                                rhs=wvt16[ot], start=(ot == 0), stop=(ot == 3))
                    else:
                        for dp in range(2):
                            nc.tensor.matmul(
                                ps, lhsT=xp2[dp][:, :, P * sv:P * (sv + 1)],
                                rhs=wv8x[dp], start=(dp == 0), stop=(dp == 1),
                                perf_mode=DR)
                    tp, tt = sv // 2, sv % 2
                    nc.scalar.copy(out=vt8[tp][:, tt, :], in_=ps)
                    if kb <= 1:
                        # V8r residual only feeds slots 0-1 (query rows < 1024)
                        nc.vector.tensor_sub(out=vt8r[tp][:, tt, :], in0=ps,
                                             in1=vt8[tp][:, tt, :])
                    if kb == 0:
                        nc.vector.tensor_copy(out=vt16[sv], in_=ps)

            def finalize(j, h, av, den):
                recip = smallp.tile([P, 1], F32, name="recip", tag="recip")
                nc.vector.reciprocal(out=recip, in_=den)
                osb = outsp.tile([P, D], BF, name="osb", tag="osb")
                nc.vector.tensor_scalar_mul(osb, av, recip)
                m = 2 * j + h
                nc.sync.dma_start(out=out_d[P * m:P * (m + 1), :], in_=osb)

            def emit_slot0():
                # all-bf16 slot for query rows < 512
                qsl0 = slice(0, 256)
                pts = []
                for tp in range(2):
                    lastp = tp == 1
                    w, qoff = (P, P) if lastp else (256, 0)
                    st = st_ps.tile([P, 2, 256], F32, name="st", tag="st")
                    for tt in range(2):
                        t = 2 * tp + tt
                        for ot in range(4):
                            nc.tensor.matmul(
                                st[:, tt, qoff:qoff + w],
                                lhsT=xt16[ot][:, P * t:P * (t + 1)],
                                rhs=at16[ot][:, qoff:qoff + w],
                                start=(ot == 0), stop=(ot == 3))
                    pt = pt16p.tile([P, 2, 256], BF, name="pt16", tag="pt16")
                    nc.scalar.activation(out=pt[:, :, qoff:qoff + w],
                                         in_=st[:, :, qoff:qoff + w],
                                         func=Exp, scale=SCALE, bias=nshift)
                    h = tp
                    for tt in range(2):
                        nc.gpsimd.tensor_mul(
                            out=pt[:, tt, P * h:P * (h + 1)],
                            in0=pt[:, tt, P * h:P * (h + 1)],
                            in1=mask16[:, tt, :])
                    pts.append(pt)
                for h in (1, 0):
                    nt = 2 * h + 2
                    den = qkv_ps.tile([P, 1], F32, name=f"den{h}", tag="qkv_ps")
                    av = av_ps.tile([P, D], F32, name=f"av{h}", tag="av")
                    for t in range(nt):
                        lhs = pts[t // 2][:, t % 2, P * h:P * (h + 1)]
                        first, last = (t == 0), (t == nt - 1)
                        nc.tensor.matmul(den, lhsT=lhs, rhs=ones16, start=first,
                                         stop=last)
                        nc.tensor.matmul(av, lhsT=lhs, rhs=vt16[t],
                                         start=first, stop=last)
                    finalize(0, h, av, den)

            def emit_slot(j):
                qb = j // 2
                qb_off = 256 * (j % 2)
                pts = []
                for tp in range(2 * j + 2):
                    lastp = tp == 2 * j + 1
                    w, qoff = (P, P) if lastp else (256, 0)
                    qsl = slice(qb_off + qoff, qb_off + qoff + w)
                    st = st_ps.tile([P, 2, 256], F32, name="st", tag="st")
                    for tt in range(2):
                        t = 2 * tp + tt
                        i = 0
                        for dp in range(2):
                            for xs, as_ in ((xp2[dp], a8x[dp][qb]),
                                            (xp2r[dp], a8x[dp][qb]),
                                            (xp2[dp], a8rx[dp][qb])):
                                nc.tensor.matmul(
                                    st[:, tt, qoff:qoff + w],
                                    lhsT=xs[:, :, P * t:P * (t + 1)],
                                    rhs=as_[:, :, qsl],
                                    start=(i == 0), stop=(i == 5),
                                    perf_mode=DR)
                                i += 1
                    pt = ptp.tile([P, 2, 256], F8, name="pt8", tag="pt8")
                    nc.scalar.activation(out=pt[:, :, qoff:qoff + w],
                                         in_=st[:, :, qoff:qoff + w],
                                         func=Exp, scale=SCALE, bias=nshift)
                    if tp >= 2 * j:
                        h = tp - 2 * j
                        for tt in range(2):
                            nc.gpsimd.tensor_mul(
                                out=pt[:, tt, P * h:P * (h + 1)],
                                in0=pt[:, tt, P * h:P * (h + 1)],
                                in1=mask8[:, tt, :])
                    pts.append(pt)
                for h in (1, 0):
                    ntp = 2 * j + h + 1
                    den = qkv_ps.tile([P, 1], F32, name=f"den{h}", tag="qkv_ps")
                    av = av_ps.tile([P, D], F32, name=f"av{h}", tag="av")
                    for tp in range(ntp):
                        lhs = pts[tp][:, :, P * h:P * (h + 1)]
                        first, last = (tp == 0), (tp == ntp - 1)
                        nc.tensor.matmul(den, lhsT=lhs, rhs=ones8, start=first,
                                         stop=last, perf_mode=DR)
                        if j == 1:
                            nc.tensor.matmul(av, lhsT=lhs, rhs=vt8[tp],
                                             start=first, stop=False,
                                             perf_mode=DR)
                            nc.tensor.matmul(av, lhsT=lhs, rhs=vt8r[tp],
                                             start=False, stop=last,
                                             perf_mode=DR)
                        else:
                            nc.tensor.matmul(av, lhsT=lhs, rhs=vt8[tp],
                                             start=first, stop=last,
                                             perf_mode=DR)
                    finalize(j, h, av, den)

            emit_c()
            for kb in range(8):
                if kb == 0:
                    emit_a(0)
                emit_kv(kb)
                if kb == 0:
                    emit_a(1)
                    emit_slot0()
                else:
                    emit_slot(kb)
                    if kb in (2, 4):
                        emit_a(kb // 2 + 1)

    return nc


def _qidx(p):
    tiles = [2 * m + p for m in range(16)]
    return np.concatenate([np.arange(P * t, P * t + P) for t in tiles])


def _masks(p, dt):
    tri = (np.arange(P)[:, None] <= np.arange(P)[None, :]).astype(np.float32)
    m = np.zeros((P, 2, P), np.float32)
    if p == 0:
        m[:, 0, :] = tri
    else:
        m[:, 0, :] = 1.0
        m[:, 1, :] = tri
    return m.astype(dt)


def kernel(input, w_query, w_key, w_value):
    global _BUILT, LAST_RESULT
    from concourse.bass_utils import run_bass_kernel_spmd

    if _BUILT is None:
        _BUILT = _build_nc()
        if not _BUILT.is_finalized():
            _BUILT.finalize()
    nc = _BUILT

    x = np.asarray(input, dtype=np.float32)
    wq16 = np.ascontiguousarray(np.asarray(w_query, np.float32)).astype(bf16)
    wk16 = np.ascontiguousarray(np.asarray(w_key, np.float32)).astype(bf16)
    wvt = np.ascontiguousarray(np.asarray(w_value, np.float32).T).astype(bf16)
    wv8 = wvt.astype(f8)
    wv8x = np.stack([wv8.reshape(4, P, D)[2 * dp:2 * dp + 2].transpose(1, 0, 2)
                     for dp in range(2)])          # [2, P, 2, D]
    qidx = {p: _qidx(p) for p in (0, 1)}
    mask8 = {p: _masks(p, f8) for p in (0, 1)}
    mask16 = {p: _masks(p, bf16) for p in (0, 1)}

    per_batch = []
    for b in range(B):
        xt = np.ascontiguousarray(x[b].astype(bf16).T)          # [D, S] bf16
        xt32 = xt.astype(np.float32)
        x8 = xt.astype(f8)
        x8r = (xt32 - x8.astype(np.float32)).astype(f8)
        x8t = x8.reshape(4, P, S)
        x8rt = x8r.reshape(4, P, S)
        xp2 = np.stack([x8t[2 * dp:2 * dp + 2].transpose(1, 0, 2)
                        for dp in range(2)])                    # [2, P, 2, S]
        xp2r = np.stack([x8rt[2 * dp:2 * dp + 2].transpose(1, 0, 2)
                         for dp in range(2)])
        xt16 = np.ascontiguousarray(xt.reshape(4, P, S)[:, :, :512])
        per_batch.append((xt, xp2, xp2r, xt16))

    in_maps = []
    for c in range(8):
        b, p = c // 2, c % 2
        xt, xp2, xp2r, xt16 = per_batch[b]
        xtq = xt[:, qidx[p]]
        xtq32 = xtq.astype(np.float32)
        xq8 = xtq.astype(f8)
        xq8r = (xtq32 - xq8.astype(np.float32)).astype(f8)
        def pair(a):
            r = a.reshape(4, P, SQ)
            return np.stack([r[2 * i:2 * i + 2].transpose(1, 0, 2)
                             for i in range(2)])
        in_maps.append({
            "xp2": xp2, "xp2r": xp2r, "xt16": xt16,
            "xq8x": pair(xq8), "xq8rx": pair(xq8r),
            "wq": wq16.reshape(4, P, D), "wk": wk16.reshape(4, P, D),
            "wv8x": wv8x, "wvt16": wvt.reshape(4, P, D),
            "mask8": mask8[p], "mask16": mask16[p],
        })

    res = run_bass_kernel_spmd(nc, in_maps, core_ids=list(range(8)),
                               trace=TRACE)
    LAST_RESULT = res

    out = np.empty((B, S, D), np.float32)
    for c in range(8):
        b, p = c // 2, c % 2
        out[b][qidx[p]] = res.results[c]["out"].astype(np.float32)
    return out


# revision 36
# speedup vs baseline: 1.0012x; 1.0012x over previous
"""Causal single-head attention (B=4, S=4096, D=512, fp32) on 8 Trainium2 cores.

Sharding: core c -> (batch b = c//2, parity p = c%2). Each core handles the 16
query tiles {2m+p} of its batch (parity-interleaved for causal balance; every
core runs the identical SPMD program, parity lives only in input data).

All heavy matmuls run as fp8-e4m3 DoubleRow (contracts 256/instr at 0.5
cyc/row vs bf16's 128 at 1.0 = 4x per MAC), with fp8+fp8-residual operand
splits recovering bf16-level precision where softmax is sensitive:
  C  = Wq^T Wk                 bf16 matmul; stored fp8 + residual (ct8x/ct8rx)
  A  = C^T Xq^T                3-term DR (c8*xq8 + c8r*xq8 + c8*xq8r ~ exact);
                               stored fp8 + residual (a8x/a8rx) + bf16 (slot 0)
  V  = X Wv^T                  tiles 0-3 bf16; tiles 4-31 plain DR (x8*wv8);
                               stored fp8 (vt8) + residual (vt8r, tiles 0-7)
  S^T = X^T A  per k-tile      3-term DR: x8*A8 + x8r*A8 + x8*A8r with
                               cross-d-tile ko-pairing -> bf16-equivalent
                               precision at 0.75x bf16 cost; slot 0 pure bf16
  P   = exp(S*scale - 4.5)     ScalarE -> fp8 (slot 0: bf16); causal masking
                               via 0/1 multiplies on the 4 diagonal-band
                               128x128 blocks per slot (GpSimd)
  out = P V / P.1              DR AV (+ V-residual term for slots 0-1, i.e.
                               rows < 1024) + ones-matmul denominator
Query rows < 512 (slot 0) take an all-bf16 path and rows < 1024 keep the
V-storage residual, because low-entropy early softmax rows amplify fp8 noise.
Input DMAs are chunked in first-consumer order; output is DMA'd bf16 and
upcast on host. Fits the TimelineSim cost model: matmul cost = out-free-size
x cycles/row, DoubleRow 0.5 cyc/row, PE p-state ramp warmed by scratch MMs.
"""

import numpy as np
import ml_dtypes

B, S, D = 4, 4096, 512
P = 128
SQ = S // 2          # local query rows per core
SCALE = float(1.0 / np.sqrt(D))
SHIFT = 4.5          # exp shift: keeps P in fp8-e4m3 range (max score ~8.4)
bf16 = ml_dtypes.bfloat16
f8 = ml_dtypes.float8_e4m3

_BUILT = None
TRACE = False        # set True from a test harness to collect an NTFF trace
LAST_RESULT = None   # BassKernelResults of the most recent run (for profiling)


def _build_nc():
    import concourse.bass as bass
    import concourse.tile as tile
    from concourse import bacc, mybir

    BF = mybir.dt.bfloat16
    F8 = mybir.dt.float8e4
    F32 = mybir.dt.float32
    Exp = mybir.ActivationFunctionType.Exp
    DR = mybir.MatmulPerfMode.DoubleRow

    nc = bacc.Bacc(trn_type="TRN2", name="causal_attn_dr")
    xp2_d = nc.declare_dram_parameter("xp2", [2, P, 2, S], F8, False)
    xp2r_d = nc.declare_dram_parameter("xp2r", [2, P, 2, S], F8, False)
    xt16_d = nc.declare_dram_parameter("xt16", [4, P, 512], BF, False)
    xq8x_d = nc.declare_dram_parameter("xq8x", [2, P, 2, SQ], F8, False)
    xq8rx_d = nc.declare_dram_parameter("xq8rx", [2, P, 2, SQ], F8, False)
    wq_d = nc.declare_dram_parameter("wq", [4, P, D], BF, False)
    wk_d = nc.declare_dram_parameter("wk", [4, P, D], BF, False)
    wv8x_d = nc.declare_dram_parameter("wv8x", [2, P, 2, D], F8, False)
    wvt16_d = nc.declare_dram_parameter("wvt16", [4, P, D], BF, False)
    mask8_d = nc.declare_dram_parameter("mask8", [P, 2, P], F8, False)
    mask16_d = nc.declare_dram_parameter("mask16", [P, 2, P], BF, False)
    out_d = nc.declare_dram_parameter("out", [SQ, D], BF, True)

    with tile.TileContext(nc) as tc:
        with (
            tc.tile_pool(name="persist", bufs=1) as persist,
            tc.tile_pool(name="ptp", bufs=24) as ptp,
            tc.tile_pool(name="pt16p", bufs=3) as pt16p,
            tc.tile_pool(name="outsp", bufs=4) as outsp,
            tc.tile_pool(name="smallp", bufs=4) as smallp,
            tc.tile_pool(name="qkv_ps", bufs=3, space="PSUM") as qkv_ps,
            tc.tile_pool(name="st_ps", bufs=3, space="PSUM") as st_ps,
            tc.tile_pool(name="av_ps", bufs=2, space="PSUM") as av_ps,
        ):
            # ---- persistent inputs, chunked in first-consumer order ----
            wq16, wk16, xt16, wvt16 = [], [], [], []
            for d in range(4):
                t = persist.tile([P, D], BF, name=f"wq{d}", tag=f"wq{d}")
                nc.sync.dma_start(out=t, in_=wq_d[d])
                wq16.append(t)
            for d in range(4):
                t = persist.tile([P, D], BF, name=f"wk{d}", tag=f"wk{d}")
                nc.sync.dma_start(out=t, in_=wk_d[d])
                wk16.append(t)
            for d in range(4):
                t = persist.tile([P, 512], BF, name=f"xt16_{d}", tag=f"xt16_{d}")
                nc.sync.dma_start(out=t, in_=xt16_d[d])
                xt16.append(t)
            for d in range(4):
                t = persist.tile([P, D], BF, name=f"wvt{d}", tag=f"wvt{d}")
                nc.sync.dma_start(out=t, in_=wvt16_d[d])
                wvt16.append(t)
            # A(0) inputs: q-block 0 slices only
            xq8x = [persist.tile([P, 2, SQ], F8, name=f"xq8x{i}", tag=f"xq8x{i}")
                    for i in range(2)]
            xq8rx = [persist.tile([P, 2, SQ], F8, name=f"xq8rx{i}",
                                  tag=f"xq8rx{i}") for i in range(2)]
            for i in range(2):
                nc.sync.dma_start(out=xq8x[i][:, :, 0:D], in_=xq8x_d[i][:, :, 0:D])
            for i in range(2):
                nc.sync.dma_start(out=xq8rx[i][:, :, 0:D],
                                  in_=xq8rx_d[i][:, :, 0:D])
            mask16 = persist.tile([P, 2, P], BF, name="mask16", tag="mask16")
            nc.sync.dma_start(out=mask16, in_=mask16_d[:])
            # fp8 X for kv1 / slot1 (k < 1024 chunks first)
            xp2 = [persist.tile([P, 2, S], F8, name=f"xp2_{dp}", tag=f"xp2_{dp}")
                   for dp in range(2)]
            xp2r = [persist.tile([P, 2, S], F8, name=f"xp2r_{dp}",
                                 tag=f"xp2r_{dp}") for dp in range(2)]
            wv8x = []
            for dp in range(2):
                nc.sync.dma_start(out=xp2[dp][:, :, 0:1024],
                                  in_=xp2_d[dp][:, :, 0:1024])
            for dp in range(2):
                t = persist.tile([P, 2, D], F8, name=f"wv8x{dp}", tag=f"wv8x{dp}")
                nc.sync.dma_start(out=t, in_=wv8x_d[dp])
                wv8x.append(t)
            mask8 = persist.tile([P, 2, P], F8, name="mask8", tag="mask8")
            nc.sync.dma_start(out=mask8, in_=mask8_d[:])
            for dp in range(2):
                nc.sync.dma_start(out=xp2r[dp][:, :, 0:1024],
                                  in_=xp2r_d[dp][:, :, 0:1024])
            # A(1) inputs
            for i in range(2):
                nc.sync.dma_start(out=xq8x[i][:, :, D:2 * D],
                                  in_=xq8x_d[i][:, :, D:2 * D])
            for i in range(2):
                nc.sync.dma_start(out=xq8rx[i][:, :, D:2 * D],
                                  in_=xq8rx_d[i][:, :, D:2 * D])
            # k in [1024, 2048) for kv2-3 / slots 2-3
            for dp in range(2):
                nc.sync.dma_start(out=xp2[dp][:, :, 1024:2048],
                                  in_=xp2_d[dp][:, :, 1024:2048])
            for dp in range(2):
                nc.sync.dma_start(out=xp2r[dp][:, :, 1024:2048],
                                  in_=xp2r_d[dp][:, :, 1024:2048])
            # remaining A q-blocks
            for i in range(2):
                nc.sync.dma_start(out=xq8x[i][:, :, 2 * D:], in_=xq8x_d[i][:, :, 2 * D:])
            for i in range(2):
                nc.sync.dma_start(out=xq8rx[i][:, :, 2 * D:],
                                  in_=xq8rx_d[i][:, :, 2 * D:])
            # k in [2048, 4096)
            for dp in range(2):
                nc.sync.dma_start(out=xp2[dp][:, :, 2048:], in_=xp2_d[dp][:, :, 2048:])
            for dp in range(2):
                nc.sync.dma_start(out=xp2r[dp][:, :, 2048:],
                                  in_=xp2r_d[dp][:, :, 2048:])

            # PE warmup: ramp the p-state clock while input DMAs stream in
            scratch = persist.tile([P, 512], BF, name="scratch", tag="scratch")
            nc.vector.memset(scratch, 0.0)
            ones8 = persist.tile([P, 2, 1], F8, name="ones8", tag="ones8")
            nc.vector.memset(ones8, 1.0)
            ones16 = persist.tile([P, 1], BF, name="ones16", tag="ones16")
            nc.vector.memset(ones16, 1.0)
            nshift = persist.tile([P, 1], F32, name="nshift", tag="nshift")
            nc.vector.memset(nshift, -SHIFT)

            for wi in range(8):
                wps = qkv_ps.tile([P, 512], F32, name="wps", tag="qkv_ps")
                nc.tensor.matmul(wps, lhsT=scratch[:, :P], rhs=scratch,
                                 start=True, stop=True)

            ct8x = [persist.tile([P, 2, D], F8, name=f"ct8x{i}", tag=f"ct8x{i}")
                    for i in range(2)]
            ct8rx = [persist.tile([P, 2, D], F8, name=f"ct8rx{i}",
                                  tag=f"ct8rx{i}") for i in range(2)]

            def emit_c():
                # C[dp, d] = sum_o Wq[o, dp] Wk[o, d]  (bf16 matmul, fp8+resid store)
                for dp in range(4):
                    ps = qkv_ps.tile([P, D], F32, name="c_ps", tag="qkv_ps")
                    for ot in range(4):
                        nc.tensor.matmul(ps,
                                         lhsT=wq16[ot][:, P * dp:P * (dp + 1)],
                                         rhs=wk16[ot], start=(ot == 0),
                                         stop=(ot == 3))
                    c8 = ct8x[dp // 2]
                    nc.vector.tensor_copy(out=c8[:, dp % 2, :], in_=ps)
                    nc.vector.tensor_sub(out=ct8rx[dp // 2][:, dp % 2, :],
                                         in0=ps, in1=c8[:, dp % 2, :])

            # persistent on-chip products
            a8x = [[persist.tile([P, 2, D], F8, name=f"a8x{dp}_{qb}",
                                 tag=f"a8x{dp}_{qb}") for qb in range(4)]
                   for dp in range(2)]
            a8rx = [[persist.tile([P, 2, D], F8, name=f"a8rx{dp}_{qb}",
                                  tag=f"a8rx{dp}_{qb}") for qb in range(4)]
                    for dp in range(2)]
            at16 = [persist.tile([P, D], BF, name=f"at16_{dt}", tag=f"at16_{dt}")
                    for dt in range(4)]
            vt8 = [persist.tile([P, 2, D], F8, name=f"vt8_{tp}", tag=f"vt8_{tp}")
                   for tp in range(16)]
            vt8r = [persist.tile([P, 2, D], F8, name=f"vt8r_{tp}", tag=f"vt8r_{tp}")
                    for tp in range(4)]
            vt16 = [persist.tile([P, D], BF, name=f"vt16_{sv}", tag=f"vt16_{sv}")
                    for sv in range(4)]

            def emit_a(qb):
                # A[d, q] = sum_dp C[dp, d] Xq^T[dp, q]
                # 3-term DoubleRow: c8*xq8 + c8r*xq8 + c8*xq8r ~= bf16 exact
                qsl = slice(D * qb, D * (qb + 1))
                for dt in range(4):
                    ps = qkv_ps.tile([P, D], F32, name="a_ps", tag="qkv_ps")
                    i = 0
                    for dpp in range(2):
                        for cs, xs in ((ct8x, xq8x), (ct8rx, xq8x),
                                       (ct8x, xq8rx)):
                            nc.tensor.matmul(
                                ps, lhsT=cs[dpp][:, :, P * dt:P * (dt + 1)],
                                rhs=xs[dpp][:, :, qsl],
                                start=(i == 0), stop=(i == 5), perf_mode=DR)
                            i += 1
                    a8 = a8x[dt // 2][qb]
                    nc.vector.tensor_copy(out=a8[:, dt % 2, :], in_=ps)
                    nc.vector.tensor_sub(out=a8rx[dt // 2][qb][:, dt % 2, :],
                                         in0=ps, in1=a8[:, dt % 2, :])
                    if qb == 0:
                        nc.scalar.copy(out=at16[dt], in_=ps)

            def emit_kv(kb):
                # V[s, o] = sum_d X[s, d] Wv[o, d]
                for sv in range(4 * kb, 4 * kb + 4):
                    ps = qkv_ps.tile([P, D], F32, name="v_ps", tag="qkv_ps")
                    if kb == 0:
                        for ot in range(4):
                            nc.tensor.matmul(
                                ps, lhsT=xt16[ot][:, P * sv:P * (sv + 1)],
                                rhs=wvt16[ot], start=(ot == 0), stop=(ot == 3))
                    else:
                        for dp in range(2):
                            nc.tensor.matmul(
                                ps, lhsT=xp2[dp][:, :, P * sv:P * (sv + 1)],
                                rhs=wv8x[dp], start=(dp == 0), stop=(dp == 1),
                                perf_mode=DR)
                    tp, tt = sv // 2, sv % 2
                    nc.scalar.copy(out=vt8[tp][:, tt, :], in_=ps)
                    if kb <= 1:
                        # V8r residual only feeds slots 0-1 (query rows < 1024)
                        nc.vector.tensor_sub(out=vt8r[tp][:, tt, :], in0=ps,
                                             in1=vt8[tp][:, tt, :])
                    if kb == 0:
                        nc.vector.tensor_copy(out=vt16[sv], in_=ps)

            def finalize(j, h, av, den):
                recip = smallp.tile([P, 1], F32, name="recip", tag="recip")
                nc.vector.reciprocal(out=recip, in_=den)
                osb = outsp.tile([P, D], BF, name="osb", tag="osb")
                nc.vector.tensor_scalar_mul(osb, av, recip)
                m = 2 * j + h
                nc.sync.dma_start(out=out_d[P * m:P * (m + 1), :], in_=osb)

            def emit_slot0():
                # all-bf16 slot for query rows < 512
                qsl0 = slice(0, 256)
                pts = []
                for tp in range(2):
                    lastp = tp == 1
                    w, qoff = (P, P) if lastp else (256, 0)
                    st = st_ps.tile([P, 2, 256], F32, name="st", tag="st")
                    for tt in range(2):
                        t = 2 * tp + tt
                        for ot in range(4):
                            nc.tensor.matmul(
                                st[:, tt, qoff:qoff + w],
                                lhsT=xt16[ot][:, P * t:P * (t + 1)],
                                rhs=at16[ot][:, qoff:qoff + w],
                                start=(ot == 0), stop=(ot == 3))
                    pt = pt16p.tile([P, 2, 256], BF, name="pt16", tag="pt16")
                    nc.scalar.activation(out=pt[:, :, qoff:qoff + w],
                                         in_=st[:, :, qoff:qoff + w],
                                         func=Exp, scale=SCALE, bias=nshift)
                    h = tp
                    for tt in range(2):
                        nc.gpsimd.tensor_mul(
                            out=pt[:, tt, P * h:P * (h + 1)],
                            in0=pt[:, tt, P * h:P * (h + 1)],
                            in1=mask16[:, tt, :])
                    pts.append(pt)
                for h in (1, 0):
                    nt = 2 * h + 2
                    den = qkv_ps.tile([P, 1], F32, name=f"den{h}", tag="qkv_ps")
                    av = av_ps.tile([P, D], F32, name=f"av{h}", tag="av")
                    for t in range(nt):
                        lhs = pts[t // 2][:, t % 2, P * h:P * (h + 1)]
                        first, last = (t == 0), (t == nt - 1)
                        nc.tensor.matmul(den, lhsT=lhs, rhs=ones16, start=first,
                                         stop=last)
                        nc.tensor.matmul(av, lhsT=lhs, rhs=vt16[t],
                                         start=first, stop=last)
                    finalize(0, h, av, den)

            def emit_slot(j):
                qb = j // 2
                qb_off = 256 * (j % 2)
                pts = []
                for tp in range(2 * j + 2):
                    lastp = tp == 2 * j + 1
                    w, qoff = (P, P) if lastp else (256, 0)
                    qsl = slice(qb_off + qoff, qb_off + qoff + w)
                    st = st_ps.tile([P, 2, 256], F32, name="st", tag="st")
                    for tt in range(2):
                        t = 2 * tp + tt
                        i = 0
                        for dp in range(2):
                            for xs, as_ in ((xp2[dp], a8x[dp][qb]),
                                            (xp2r[dp], a8x[dp][qb]),
                                            (xp2[dp], a8rx[dp][qb])):
                                nc.tensor.matmul(
                                    st[:, tt, qoff:qoff + w],
                                    lhsT=xs[:, :, P * t:P * (t + 1)],
                                    rhs=as_[:, :, qsl],
                                    start=(i == 0), stop=(i == 5),
                                    perf_mode=DR)
                                i += 1
                    pt = ptp.tile([P, 2, 256], F8, name="pt8", tag="pt8")
                    nc.scalar.activation(out=pt[:, :, qoff:qoff + w],
                                         in_=st[:, :, qoff:qoff + w],
                                         func=Exp, scale=SCALE, bias=nshift)
                    if tp >= 2 * j:
                        h = tp - 2 * j
                        for tt in range(2):
                            nc.gpsimd.tensor_mul(
                                out=pt[:, tt, P * h:P * (h + 1)],
                                in0=pt[:, tt, P * h:P * (h + 1)],
                                in1=mask8[:, tt, :])
                    pts.append(pt)
                for h in (1, 0):
                    ntp = 2 * j + h + 1
                    den = qkv_ps.tile([P, 1], F32, name=f"den{h}", tag="qkv_ps")
                    av = av_ps.tile([P, D], F32, name=f"av{h}", tag="av")
                    for tp in range(ntp):
                        lhs = pts[tp][:, :, P * h:P * (h + 1)]
                        first, last = (tp == 0), (tp == ntp - 1)
                        nc.tensor.matmul(den, lhsT=lhs, rhs=ones8, start=first,
                                         stop=last, perf_mode=DR)
                        if j == 1:
                            nc.tensor.matmul(av, lhsT=lhs, rhs=vt8[tp],
                                             start=first, stop=False,
                                             perf_mode=DR)
                            nc.tensor.matmul(av, lhsT=lhs, rhs=vt8r[tp],
                                             start=False, stop=last,
                                             perf_mode=DR)
                        else:
                            nc.tensor.matmul(av, lhsT=lhs, rhs=vt8[tp],
                                             start=first, stop=last,
                                             perf_mode=DR)
                    finalize(j, h, av, den)

            emit_c()
            for kb in range(8):
                if kb == 0:
                    emit_a(0)
                if kb <= 1:
                    emit_kv(kb)
                if kb == 0:
                    emit_a(1)
                elif kb >= 2:
                    emit_kv(kb)
                if kb == 0:
                    emit_slot0()
                else:
                    emit_slot(kb)
                    if kb in (2, 4):
                        emit_a(kb // 2 + 1)

    return nc


def _qidx(p):
    tiles = [2 * m + p for m in range(16)]
    return np.concatenate([np.arange(P * t, P * t + P) for t in tiles])


def _masks(p, dt):
    tri = (np.arange(P)[:, None] <= np.arange(P)[None, :]).astype(np.float32)
    m = np.zeros((P, 2, P), np.float32)
    if p == 0:
        m[:, 0, :] = tri
    else:
        m[:, 0, :] = 1.0
        m[:, 1, :] = tri
    return m.astype(dt)


def kernel(input, w_query, w_key, w_value):
    global _BUILT, LAST_RESULT
    from concourse.bass_utils import run_bass_kernel_spmd

    if _BUILT is None:
        _BUILT = _build_nc()
        if not _BUILT.is_finalized():
            _BUILT.finalize()
    nc = _BUILT

    x = np.asarray(input, dtype=np.float32)
    wq16 = np.ascontiguousarray(np.asarray(w_query, np.float32)).astype(bf16)
    wk16 = np.ascontiguousarray(np.asarray(w_key, np.float32)).astype(bf16)
    wvt = np.ascontiguousarray(np.asarray(w_value, np.float32).T).astype(bf16)
    wv8 = wvt.astype(f8)
    wv8x = np.stack([wv8.reshape(4, P, D)[2 * dp:2 * dp + 2].transpose(1, 0, 2)
                     for dp in range(2)])          # [2, P, 2, D]
    qidx = {p: _qidx(p) for p in (0, 1)}
    mask8 = {p: _masks(p, f8) for p in (0, 1)}
    mask16 = {p: _masks(p, bf16) for p in (0, 1)}

    per_batch = []
    for b in range(B):
        xt = np.ascontiguousarray(x[b].astype(bf16).T)          # [D, S] bf16
        xt32 = xt.astype(np.float32)
        x8 = xt.astype(f8)
        x8r = (xt32 - x8.astype(np.float32)).astype(f8)
        x8t = x8.reshape(4, P, S)
        x8rt = x8r.reshape(4, P, S)
        xp2 = np.stack([x8t[2 * dp:2 * dp + 2].transpose(1, 0, 2)
                        for dp in range(2)])                    # [2, P, 2, S]
        xp2r = np.stack([x8rt[2 * dp:2 * dp + 2].transpose(1, 0, 2)
                         for dp in range(2)])
        xt16 = np.ascontiguousarray(xt.reshape(4, P, S)[:, :, :512])
        per_batch.append((xt, xp2, xp2r, xt16))

    in_maps = []
    for c in range(8):
        b, p = c // 2, c % 2
        xt, xp2, xp2r, xt16 = per_batch[b]
        xtq = xt[:, qidx[p]]
        xtq32 = xtq.astype(np.float32)
        xq8 = xtq.astype(f8)
        xq8r = (xtq32 - xq8.astype(np.float32)).astype(f8)
        def pair(a):
            r = a.reshape(4, P, SQ)
            return np.stack([r[2 * i:2 * i + 2].transpose(1, 0, 2)
                             for i in range(2)])
        in_maps.append({
            "xp2": xp2, "xp2r": xp2r, "xt16": xt16,
            "xq8x": pair(xq8), "xq8rx": pair(xq8r),
            "wq": wq16.reshape(4, P, D), "wk": wk16.reshape(4, P, D),
            "wv8x": wv8x, "wvt16": wvt.reshape(4, P, D),
            "mask8": mask8[p], "mask16": mask16[p],
        })

    res = run_bass_kernel_spmd(nc, in_maps, core_ids=list(range(8)),
                               trace=TRACE)
    LAST_RESULT = res

    out = np.empty((B, S, D), np.float32)
    for c in range(8):
        b, p = c // 2, c % 2
        out[b][qidx[p]] = res.results[c]["out"].astype(np.float32)
    return out
